# revision 30
# baseline (speedup 1.0000x reference)
"""Trainium2 Bass kernel for BaseMultiheadAttention.

dims: B=1, V=4, S=2048, E=512, H=8, D=64 (head_dim), causal, interleaved RoPE.

Sharding (8 cores): core c -> bv index g = c//2, head-group hg = c%2
(4 heads each).  Each core computes its bv-slice's QKV projection restricted
to its 4 heads, RoPE, causal attention, and a partial output projection
(its heads' wO rows).  Host sums the two partials per bv index.

v2: all matmuls in fp16 (the fp32/float32r path compiles to fp32_mode=HIGH,
~4 PE passes per row; fp16 streams 1 row/cycle), reciprocal_approx_fast for
the softmax denominators, PSUM->SBUF drains on the idle Pool engine, fp16
DRAM I/O.  PSUM accumulation stays fp32 throughout.

Device-side layout (per core):
  xT      (512,2048) fp16  x[g]^T, e on partitions (4 chunks of 128)
  q^T/k^T (128,2048)x2 stacks: two heads per 128 partitions, d on partitions
  RoPE:   qrot = qT*cos + shuffle(qT)*sin' on DVE (fp32 from PSUM, fp16 out)
  scores  S^T (s_k on partitions, s_q free), per (j: s_q 512-block,
          i: s_k 128-tile, h: head) -> psum bank; causal diag masked by
          PRE-FILLING psum with a -60000 upper-tri bias via an identity
          matmul, then accumulating the real scores on top (start=False).
  exp     one ACT instruction per (i,j) across both heads (strided AP),
          fp16 output.
  PV      lhsT = V tile (128, 65) fp16 with a ones column -> row 64 of the
          psum output accumulates the softmax denominator for free.
  norm    denom row -> fp16, broadcast via 1-contraction matmul,
          reciprocal_approx_fast psum->SBUF, DVE mul (doubles as the
          PSUM->SBUF move of O^T).
  outproj woT chunks stationary, O^T moving; psum -> SBUF fp16 -> DRAM outT.
"""

import numpy as np

import concourse.bass as bass
import concourse.mybir as mybir
from concourse.tile import TileContext
from concourse import library_config
from concourse.bass_utils import run_bass_kernel_spmd

# ---- problem dims (hardcoded per the task contract) ----
B, V, S, E, H = 1, 4, 2048, 512, 8
D = E // H            # 64
HG = 4                # heads per core
NCORE = 8
NT = S // 128         # 16 s_k tiles
NJ = S // 512         # 4 s_q blocks
F16 = mybir.dt.float16
F32 = mybir.dt.float32

_NEG = -60000.0       # fp16-representable; exp(scale*(x+_NEG)) == 0


def _host_tables():
    pos = np.arange(S, dtype=np.float64)
    inv_freq = 1.0 / (10000.0 ** (np.arange(0, D, 2, dtype=np.float64) / D))
    freqs = pos[:, None] * inv_freq[None, :]          # (S, D/2)
    freqs = np.repeat(freqs, 2, axis=-1)              # (S, D) interleaved
    cosT = np.cos(freqs).T.astype(np.float32)         # (D, S)
    sinT = np.sin(freqs).T.astype(np.float32)
    cs = np.concatenate([cosT, cosT], axis=0)         # (128, S) two-head stack
    sn = np.concatenate([sinT, sinT], axis=0)
    # causal keep-mask for the 128-wide diag block: keep cols x >= p
    mask01 = np.triu(np.ones((128, 128), dtype=np.float16))
    return cs, sn, mask01


def _host_weights(wqkv_w, wqkv_b, wo_w, hg):
    """Per-head-group weight slices in the kernel's layouts."""
    heads = [hg * HG + h for h in range(HG)]
    cs, sn, mask01 = _host_tables()
    # feature index inside each qkv block: d*H + h  (d fastest-major: index = d*8+h)
    def rows(block, h):
        d = np.arange(D)
        return block * E + d * H + h
    Wq = np.stack([wqkv_w[rows(0, h)] for h in heads])   # (HG, D, E)
    Wk = np.stack([wqkv_w[rows(1, h)] for h in heads])
    Wv = np.stack([wqkv_w[rows(2, h)] for h in heads])
    def to_T(Wh):   # (HG, D, E) -> (E, HG*D) with col = h*64+d
        return np.transpose(Wh, (2, 0, 1)).reshape(E, HG * D).astype(np.float16)
    wqT = to_T(Wq)
    wkT = to_T(Wk)
    wvT = to_T(Wv)
    # wo rows for this head group: out feature = h_global*64 + d
    woT = np.stack([wo_w[:, (hg * HG + h) * D:(hg * HG + h + 1) * D].T
                    for h in range(HG)])                 # (HG, D, E)
    woT = woT.reshape(HG * D, E).astype(np.float16)      # (256, 512)
    sgn = np.tile(np.array([-1.0, 1.0], np.float32), D // 2)[:, None]
    snS = sn * np.concatenate([sgn, sgn], axis=0)
    return dict(wqT=wqT, wkT=wkT, wvT=wvT, woT=woT,
                cs=cs.astype(np.float16), snS=snS.astype(np.float16),
                mask01=mask01,
                vones=np.ones((128, HG), np.float16),
                ones64=np.full((1, D), -1.0, np.float16))


_MAX_WAITS = {"Matmult": 1}          # per-opcode cap; default below
_DEF_MAX_WAITS = 1


def _split_excess_waits(nc):
    """This walrus build encodes at most ~1 sync-wait per instruction
    (the tail Drain with 3 waits and matmuls with 2 fail codegen with
    'Too many sync wait commands').  Post-process the serialized BIR:
    hoist excess on_wait entries onto same-engine NoOp carriers emitted
    immediately before the instruction."""
    import orjson

    orig = nc.to_json_bytes

    def patched(_self=None):
        d = orjson.loads(orig())
        for fn in d.get("functions", []):
            for bb in fn.get("basicblocks", fn.get("blocks", [])):
                insts = bb.get("instructions")
                if insts is None:
                    continue
                out, nctr = [], 0
                for inst in insts:
                    si = inst.get("sync_info")
                    waits = (si or {}).get("on_wait") or []
                    cap = _MAX_WAITS.get(inst.get("opcode"), _DEF_MAX_WAITS)
                    if len(waits) > cap:
                        keep = waits[:cap]
                        extra = waits[cap:]
                        for w in extra:
                            nctr += 1
                            out.append({
                                "debug": inst.get("debug", 0),
                                "engine": inst["engine"],
                                "ins": [], "outs": [],
                                "name": f"{inst['name']}_w{nctr}",
                                "opcode": "NoOp",
                                "sync_info": {"on_wait": [w],
                                              "on_update": []},
                            })
                        si["on_wait"] = keep
                    out.append(inst)
                bb["instructions"] = out
        return orjson.dumps(d)

    nc.to_json_bytes = patched
    return nc


def build_nc(reps=1):
    nc = bass.Bass()
    xT = nc.declare_dram_parameter("xT", [E, S], F16, isOutput=False)
    wqT = nc.declare_dram_parameter("wqT", [E, HG * D], F16, isOutput=False)
    wkT = nc.declare_dram_parameter("wkT", [E, HG * D], F16, isOutput=False)
    wvT = nc.declare_dram_parameter("wvT", [E, HG * D], F16, isOutput=False)
    woT = nc.declare_dram_parameter("woT", [HG * D, E], F16, isOutput=False)
    cs = nc.declare_dram_parameter("cs", [128, S], F16, isOutput=False)
    snS = nc.declare_dram_parameter("snS", [128, S], F16, isOutput=False)
    mask01 = nc.declare_dram_parameter("mask01", [128, 128], F16,
                                       isOutput=False)
    vones = nc.declare_dram_parameter("vones", [128, HG], F16, isOutput=False)
    ones64 = nc.declare_dram_parameter("ones64", [1, D], F16, isOutput=False)
    outT = nc.declare_dram_parameter("outT", [E, S], F16, isOutput=True)

    JORDER = [1, 2, 3, 0]   # process the small j block last -> short tail
    SWAP = [1, 0, 3, 2, 5, 4, 7, 6, 9, 8, 11, 10, 13, 12, 15, 14,
            17, 16, 19, 18, 21, 20, 23, 22, 25, 24, 27, 26, 29, 28, 31, 30]

    with TileContext(nc) as tc, nc.allow_low_precision("fp16 kernel by design"):
      for _rep in range(reps):
        with (
            tc.tile_pool(name="const", bufs=1) as cpool,
            tc.tile_pool(name="qk", bufs=1) as qkpool,
            tc.tile_pool(name="v", bufs=1) as vpool,
            tc.tile_pool(name="pt", bufs=2) as ptpool,
            tc.tile_pool(name="on", bufs=2) as onpool,
            tc.tile_pool(name="sums", bufs=2) as spool,
            tc.tile_pool(name="oc", bufs=3) as ocpool,
        ):
            # ---- inputs: xt spread across SP/DVE/ACT queues for fast start,
            #      weights on ACT, tables on DVE ----
            xt = [cpool.tile([128, S], F16, tag=f"xt{e4}", name=f"xt{e4}")
                  for e4 in range(4)]
            xq = {0: nc.sync, 1: nc.gpsimd, 2: nc.scalar, 3: nc.sync}
            for sh in range(2):
                for e4 in range(4):
                    xq[e4].dma_start(
                        xt[e4][:, sh * 1024:(sh + 1) * 1024],
                        xT[e4 * 128:(e4 + 1) * 128, sh * 1024:(sh + 1) * 1024])
                if sh == 0:
                    wv_t = []
                    for e4 in range(4):
                        t = cpool.tile([128, HG * D], F16, tag=f"wv{e4}",
                                       name=f"wv{e4}")
                        nc.scalar.dma_start(t[:, :],
                                            wvT[e4 * 128:(e4 + 1) * 128, :])
                        wv_t.append(t)
            wq_t, wk_t = [], []
            for nm, dram, lst in (("wk", wkT, wk_t), ("wq", wqT, wq_t)):
                for e4 in range(4):
                    t = cpool.tile([128, HG * D], F16, tag=f"{nm}{e4}",
                                   name=f"{nm}{e4}")
                    nc.scalar.dma_start(t[:, :], dram[e4 * 128:(e4 + 1) * 128, :])
                    lst.append(t)
            wo_t = []
            for f2 in range(2):
                t = cpool.tile([128, E], F16, tag=f"wo{f2}", name=f"wo{f2}")
                nc.scalar.dma_start(t[:, :], woT[f2 * 128:(f2 + 1) * 128, :])
                wo_t.append(t)
            cs16 = cpool.tile([128, S], F16, tag="cs16", name="cs16")
            sn16 = cpool.tile([128, S], F16, tag="sn16", name="sn16")
            nc.gpsimd.dma_start(cs16[:, :], cs[:, :])
            nc.gpsimd.dma_start(sn16[:, :], snS[:, :])
            cs_t = cpool.tile([128, S], F32, tag="cs", name="cs_t")
            sn_t = cpool.tile([128, S], F32, tag="sn", name="sn_t")
            mask_t = cpool.tile([128, 128], F16, tag="mask01", name="mask_t")
            nc.gpsimd.dma_start(mask_t[:, :], mask01[:, :])
            ones_t = cpool.tile([1, D], F16, tag="ones64", name="ones_t")
            nc.gpsimd.dma_start(ones_t[:, :], ones64[:, :])
            nln32 = cpool.tile([64, 1], F32, tag="nln32", name="nln32")
            nc.vector.memset(nln32[:, :], float(-np.log(32.0)))

            # ======== projections + attention, one psum pool ========
            # bank map: duo (128,1024)x3 = 6 banks [q/k proj, scores pairs]
            #           b1  (128,512)x2  = 2 banks [V proj, PV accum, outproj]
            scale = 1.0 / np.sqrt(D)
            v_t = []
            qrot, krot = {}, {}
            onorm = {}
            with tc.tile_pool(name="psA", bufs=1, space="PSUM") as psA:
                def v_proj(ilist, drain):
                    for i in ilist:
                        vt = vpool.tile([128, HG * 65], F16, tag=f"v{i}",
                                        name=f"v{i}")
                        v_t.append(vt)
                        pv = psA.tile([128, 512], F32, tag="b1", bufs=2,
                                      name="pv")
                        for e4 in range(4):
                            nc.tensor.matmul(
                                pv[:, 0:HG * D],
                                xt[e4][:, i * 128:(i + 1) * 128],
                                wv_t[e4][:, :], start=(e4 == 0), stop=(e4 == 3))
                        nc.sync.dma_start(
                            vt[:, :].rearrange("p (h x) -> p h x",
                                               h=HG)[:, :, D:D + 1],
                            vones[:, :, None])
                        drain(
                            vt[:, :].rearrange("p (h x) -> p h x",
                                               h=HG)[:, :, 0:D],
                            pv[:, 0:HG * D].rearrange("p (h d) -> p h d",
                                                      h=HG))

                def project_rope(tgt, wlist, dst, st):
                    rt = qkpool.tile([128, S], F16, tag=f"{tgt}rot{st}",
                                     name=f"{tgt}rot{st}")
                    dst[st] = rt
                    fcol = st * 2 * D
                    for half in range(2):
                        s0 = half * 1024
                        pq = psA.tile([128, 1024], F32, tag="pq",
                                      bufs=1, name="pq")
                        for nb in range(2):
                            c0 = s0 + nb * 512
                            for e4 in range(4):
                                nc.tensor.matmul(
                                    pq[:, nb * 512:(nb + 1) * 512],
                                    wlist[e4][:, fcol:fcol + 128],
                                    xt[e4][:, c0:c0 + 512],
                                    start=(e4 == 0), stop=(e4 == 3))
                        t1 = ptpool.tile([128, 1024], F32, tag="ropetmp",
                                         name="ropetmp")
                        sh_t = ptpool.tile([128, 1024], F32, tag="ropesh",
                                           name="ropesh")
                        nc.vector.stream_shuffle(sh_t[:, :], pq[:, :], SWAP)
                        nc.vector.tensor_mul(
                            t1[:, :], pq[:, :], cs_t[:, s0:s0 + 1024])
                        nc.vector.tensor_mul(
                            sh_t[:, :], sh_t[:, :], sn_t[:, s0:s0 + 1024])
                        nc.vector.tensor_add(
                            rt[:, s0:s0 + 1024], sh_t[:, :], t1[:, :])

                def attention_pass(hp, jlist):
                    """Heads 2hp, 2hp+1 (stack hp).  hp=1 also runs outproj."""
                    for j in jlist:
                        po = [psA.tile([128, 512], F32, tag="b1", bufs=2,
                                       name=f"po{hh}") for hh in range(2)]
                        for i in range(4 * j + 4):
                            sc = psA.tile([128, 1024], F32, tag="duo",
                                          bufs=2, name="sc")
                            r = i - 4 * j
                            offs = 128 * r if r >= 0 else 0
                            partial = r >= 0
                            for hh in range(2):
                                nc.tensor.matmul(
                                    sc[:, hh * 512 + offs:hh * 512 + 512],
                                    krot[hp][hh * D:hh * D + D,
                                             i * 128:(i + 1) * 128],
                                    qrot[hp][hh * D:hh * D + D,
                                             j * 512 + offs:(j + 1) * 512],
                                    start=True, stop=True,
                                    tile_position=(hh * D, 0))
                            pt = ptpool.tile([128, 1024], F16, tag="pt",
                                             bufs=3, name="pt")
                            src = sc[:, :].rearrange("p (h x) -> p h x",
                                                     h=2)[:, :, offs:512]
                            dstv = pt[:, :].rearrange("p (h x) -> p h x",
                                                      h=2)[:, :, offs:512]
                            nc.scalar.activation(
                                dstv, src, mybir.ActivationFunctionType.Exp,
                                scale=float(scale))
                            if partial:
                                # zero the future (upper-tri of the diag
                                # 128-block) on the Pool engine, SBUF-only
                                for hh in range(2):
                                    dv = pt[:, hh * 512 + offs:
                                            hh * 512 + offs + 128]
                                    nc.gpsimd.tensor_mul(dv, dv, mask_t[:, :])
                            for hh in range(2):
                                h = 2 * hp + hh
                                nc.tensor.matmul(
                                    po[hh][0:65, offs:512],
                                    v_t[i][:, h * 65:h * 65 + 65],
                                    pt[:, hh * 512 + offs:hh * 512 + 512],
                                    start=(i == 0), stop=(i == 4 * j + 3))
                        # normalize into the persistent onorm chunk.
                        # Drain po -> SBUF first (frees the b1 psum ring
                        # ~2us earlier than waiting out the norm chain),
                        # then 1/denom via exp(-ln(d)): ACT Ln on the denom
                        # row, a -1s matmul broadcasts -ln(d/32) to 64
                        # partitions, ACT Exp(x - ln32) drains it as 1/d.
                        onj = onpool.tile([128, 512], F16, tag=f"on{j}{hp}",
                                          name=f"on{j}{hp}")
                        onorm[(j, hp)] = onj
                        poS = [spool.tile([65, 512], F32, tag=f"poS{hh}",
                                          name=f"poS{hh}") for hh in range(2)]
                        nld = [spool.tile([1, 512], F16, tag=f"srow{hh}",
                                          name=f"nld{hh}") for hh in range(2)]
                        rbs = [spool.tile([64, 512], F32, tag=f"rb{hh}",
                                          name=f"rb{hh}") for hh in range(2)]
                        for hh in range(2):
                            nc.vector.tensor_copy(poS[hh][:, :], po[hh][0:65, :])
                        for hh in range(2):
                            nc.scalar.activation(
                                nld[hh][:, :], poS[hh][64:65, :],
                                mybir.ActivationFunctionType.Ln,
                                scale=float(1.0 / 32.0))
                        bc = psA.tile([128, 1024], F32, tag="pq",
                                      bufs=1, name="bc")
                        for hh in range(2):
                            nc.tensor.matmul(
                                bc[0:D, hh * 512:hh * 512 + 512],
                                ones_t[:, :], nld[hh][:, :],
                                start=True, stop=True)
                        for hh in range(2):
                            nc.scalar.activation(
                                rbs[hh][:, :],
                                bc[0:D, hh * 512:hh * 512 + 512],
                                mybir.ActivationFunctionType.Exp,
                                bias=nln32[:, :])
                        for hh in range(2):
                            nc.vector.tensor_mul(
                                onj[hh * D:hh * D + D, :],
                                poS[hh][0:D, :], rbs[hh][:, :])
                        if hp == 1:
                            for eb in range(4):
                                pp = psA.tile([128, 512], F32, tag="b1",
                                              bufs=2, name="pp")
                                for f2 in range(2):
                                    nc.tensor.matmul(
                                        pp[:, :],
                                        wo_t[f2][:, eb * 128:
                                                 (eb + 1) * 128],
                                        onorm[(j, f2)][:, :],
                                        start=(f2 == 0), stop=(f2 == 1))
                                oc = ocpool.tile([128, 512], F16, tag="oc",
                                                 name="oc")
                                nc.vector.tensor_copy(oc[:, :], pp[:, :])
                                nc.sync.dma_start(
                                    outT[eb * 128:(eb + 1) * 128,
                                         j * 512:(j + 1) * 512],
                                    oc[:, :])

                v_proj(range(8), nc.scalar.copy)
                nc.vector.tensor_copy(cs_t[:, :], cs16[:, :])
                nc.vector.tensor_copy(sn_t[:, :], sn16[:, :])
                project_rope("k", wk_t, krot, 0)
                project_rope("q", wq_t, qrot, 0)
                attention_pass(0, [1])
                v_proj(range(8, NT), nc.vector.tensor_copy)
                project_rope("k", wk_t, krot, 1)
                attention_pass(0, [2])
                project_rope("q", wq_t, qrot, 1)
                attention_pass(0, [3, 0])
                attention_pass(1, JORDER)
    return _split_excess_waits(nc)


_NC_CACHE = {}


def _get_nc(reps=1):
    if reps not in _NC_CACHE:
        _NC_CACHE[reps] = build_nc(reps)
    return _NC_CACHE[reps]


_RUNNER_CACHE = {}


def _get_runner(nc, n_cores):
    """Clone of bass2jax.run_bass_via_pjrt's multi-core path with the
    jitted callable cached so repeat calls skip retracing."""
    key = id(nc)
    if key in _RUNNER_CACHE:
        return _RUNNER_CACHE[key]
    import jax
    from jax.sharding import Mesh, PartitionSpec
    from jax.experimental.shard_map import shard_map
    from concourse import bass2jax as b2j

    b2j.install_neuronx_cc_hook()
    partition_name = (nc.partition_id_tensor.name
                      if nc.partition_id_tensor else None)
    in_names, out_names, out_avals, zero_outs = [], [], [], []
    for alloc in nc.m.functions[0].allocations:
        if not isinstance(alloc, mybir.MemoryLocationSet):
            continue
        name = alloc.memorylocations[0].name
        if alloc.kind == "ExternalInput":
            if name != partition_name:
                in_names.append(name)
        elif alloc.kind == "ExternalOutput":
            shape = tuple(alloc.tensor_shape)
            dtype = mybir.dt.np(alloc.dtype)
            out_names.append(name)
            out_avals.append(jax.core.ShapedArray(shape, dtype))
            zero_outs.append(np.zeros(shape, dtype))
    n_params = len(in_names)
    n_outs = len(out_avals)
    in_names_all = list(in_names) + list(out_names)
    if partition_name is not None:
        in_names_all.append(partition_name)
    donate = tuple(range(n_params, n_params + n_outs))

    def _body(*args):
        operands = list(args)
        if partition_name is not None:
            operands.append(b2j.partition_id_tensor())
        outs = b2j._bass_exec_p.bind(
            *operands,
            out_avals=tuple(out_avals),
            in_names=tuple(in_names_all),
            out_names=tuple(out_names),
            lowering_input_output_aliases=(),
            sim_require_finite=True,
            sim_require_nnan=True,
            nc=nc,
        )
        return tuple(outs)

    devices = jax.devices()[:n_cores]
    mesh = Mesh(np.asarray(devices), ("core",))
    in_specs = (PartitionSpec("core"),) * (n_params + n_outs)
    out_specs = (PartitionSpec("core"),) * len(out_names)
    sharded = jax.jit(
        shard_map(_body, mesh=mesh, in_specs=in_specs, out_specs=out_specs,
                  check_rep=False),
        donate_argnums=donate, keep_unused=True)

    def run(in_maps):
        gins = [np.concatenate([np.asarray(m[name]) for m in in_maps], axis=0)
                for name in in_names]
        gzeros = [np.concatenate([z] * n_cores, axis=0) for z in zero_outs]
        outs = sharded(*gins, *gzeros)
        res = []
        for c in range(n_cores):
            res.append({})
        for i, name in enumerate(out_names):
            arr = np.asarray(outs[i])
            per = arr.shape[0] // n_cores
            for c in range(n_cores):
                res[c][name] = arr[c * per:(c + 1) * per]
        return res

    _RUNNER_CACHE[key] = run
    return run


def _make_in_maps(x, wqkv_w, wqkv_b, wo_w):
    in_maps = []
    wcache = {}
    for c in range(NCORE):
        g, hg = c // 2, c % 2
        if hg not in wcache:
            wcache[hg] = _host_weights(wqkv_w, wqkv_b, wo_w, hg)
        wd = wcache[hg]
        in_maps.append(dict(
            xT=np.ascontiguousarray(x[g].T.astype(np.float16)),
            wqT=wd["wqT"], wkT=wd["wkT"],
            wvT=wd["wvT"], woT=wd["woT"], cs=wd["cs"], snS=wd["snS"],
            mask01=wd["mask01"], vones=wd["vones"],
            ones64=wd["ones64"],
        ))
    return in_maps


def kernel(layer_idx=None, inputs=None, wqkv_w=None, wqkv_b=None,
           wo_w=None, wo_b=None):
    inputs = np.asarray(inputs, dtype=np.float32)
    wqkv_w = np.asarray(wqkv_w, dtype=np.float32)
    wqkv_b = np.asarray(wqkv_b, dtype=np.float32)
    wo_w = np.asarray(wo_w, dtype=np.float32)
    wo_b = np.asarray(wo_b, dtype=np.float32)
    assert not np.any(wqkv_b), "nonzero wqkv_b not supported by this kernel build"

    x = inputs.reshape(B * V, S, E)
    nc = _get_nc()
    in_maps = _make_in_maps(x, wqkv_w, wqkv_b, wo_w)

    run = _get_runner(nc, NCORE)
    outs = run(in_maps)
    y = np.empty((B * V, S, E), dtype=np.float32)
    for g in range(B * V):
        acc = (outs[2 * g]["outT"].astype(np.float32)
               + outs[2 * g + 1]["outT"].astype(np.float32))   # (E, S)
        y[g] = acc.T
    y += wo_b[None, None, :]
    return y.reshape(B, V, S, E)


# revision 31
# speedup vs baseline: 1.2030x; 1.2030x over previous
"""Trainium2 Bass kernel for BaseMultiheadAttention.

dims: B=1, V=4, S=2048, E=512, H=8, D=64 (head_dim), causal, interleaved RoPE.

Sharding (8 cores): core c -> bv index g = c//2, head-group hg = c%2
(4 heads each).  Each core computes its bv-slice's QKV projection restricted
to its 4 heads, RoPE, causal attention, and a partial output projection
(its heads' wO rows).  Host sums the two partials per bv index.

v2: all matmuls in fp16 (the fp32/float32r path compiles to fp32_mode=HIGH,
~4 PE passes per row; fp16 streams 1 row/cycle), reciprocal_approx_fast for
the softmax denominators, PSUM->SBUF drains on the idle Pool engine, fp16
DRAM I/O.  PSUM accumulation stays fp32 throughout.

Device-side layout (per core):
  xT      (512,2048) fp16  x[g]^T, e on partitions (4 chunks of 128)
  q^T/k^T (128,2048)x2 stacks: two heads per 128 partitions, d on partitions
  RoPE:   qrot = qT*cos + shuffle(qT)*sin' on DVE (fp32 from PSUM, fp16 out)
  scores  S^T (s_k on partitions, s_q free), per (j: s_q 512-block,
          i: s_k 128-tile, h: head) -> psum bank; causal diag masked by
          PRE-FILLING psum with a -60000 upper-tri bias via an identity
          matmul, then accumulating the real scores on top (start=False).
  exp     one ACT instruction per (i,j) across both heads (strided AP),
          fp16 output.
  PV      lhsT = V tile (128, 65) fp16 with a ones column -> row 64 of the
          psum output accumulates the softmax denominator for free.
  norm    denom row -> fp16, broadcast via 1-contraction matmul,
          reciprocal_approx_fast psum->SBUF, DVE mul (doubles as the
          PSUM->SBUF move of O^T).
  outproj woT chunks stationary, O^T moving; psum -> SBUF fp16 -> DRAM outT.
"""

import numpy as np

import concourse.bass as bass
import concourse.mybir as mybir
from concourse.tile import TileContext

# ---- problem dims (hardcoded per the task contract) ----
B, V, S, E, H = 1, 4, 2048, 512, 8
D = E // H            # 64
HG = 4                # heads per core
NCORE = 8
NT = S // 128         # 16 s_k tiles
NJ = S // 512         # 4 s_q blocks
F16 = mybir.dt.float16
F32 = mybir.dt.float32



def _host_tables():
    pos = np.arange(S, dtype=np.float64)
    inv_freq = 1.0 / (10000.0 ** (np.arange(0, D, 2, dtype=np.float64) / D))
    freqs = pos[:, None] * inv_freq[None, :]          # (S, D/2)
    freqs = np.repeat(freqs, 2, axis=-1)              # (S, D) interleaved
    cosT = np.cos(freqs).T.astype(np.float32)         # (D, S)
    sinT = np.sin(freqs).T.astype(np.float32)
    cs = np.concatenate([cosT, cosT], axis=0)         # (128, S) two-head stack
    sn = np.concatenate([sinT, sinT], axis=0)
    # causal keep-mask for the 128-wide diag block: keep cols x >= p
    mask01 = np.triu(np.ones((128, 128), dtype=np.float16))
    return cs, sn, mask01


def _host_weights(wqkv_w, wqkv_b, wo_w, hg):
    """Per-head-group weight slices in the kernel's layouts."""
    heads = [hg * HG + h for h in range(HG)]
    cs, sn, mask01 = _host_tables()
    # feature index inside each qkv block: d*H + h  (d fastest-major: index = d*8+h)
    def rows(block, h):
        d = np.arange(D)
        return block * E + d * H + h
    Wq = np.stack([wqkv_w[rows(0, h)] for h in heads])   # (HG, D, E)
    Wk = np.stack([wqkv_w[rows(1, h)] for h in heads])
    Wv = np.stack([wqkv_w[rows(2, h)] for h in heads])
    def to_T(Wh):   # (HG, D, E) -> (E, HG*D) with col = h*64+d
        return np.transpose(Wh, (2, 0, 1)).reshape(E, HG * D).astype(np.float16)
    wqT = to_T(Wq)
    wkT = to_T(Wk)
    wvT = to_T(Wv)
    # wo rows for this head group: out feature = h_global*64 + d
    woT = np.stack([wo_w[:, (hg * HG + h) * D:(hg * HG + h + 1) * D].T
                    for h in range(HG)])                 # (HG, D, E)
    woT = woT.reshape(HG * D, E).astype(np.float16)      # (256, 512)
    sgn = np.tile(np.array([-1.0, 1.0], np.float32), D // 2)[:, None]
    snS = sn * np.concatenate([sgn, sgn], axis=0)
    return dict(wqT=wqT, wkT=wkT, wvT=wvT, woT=woT,
                cs=cs.astype(np.float16), snS=snS.astype(np.float16),
                mask01=mask01,
                vones=np.ones((128, HG), np.float16),
                ones64=np.full((1, D), -1.0, np.float16))


_MAX_WAITS = {"Matmult": 1}          # per-opcode cap; default below
_DEF_MAX_WAITS = 1


def _split_excess_waits(nc):
    """This walrus build encodes at most ~1 sync-wait per instruction
    (the tail Drain with 3 waits and matmuls with 2 fail codegen with
    'Too many sync wait commands').  Post-process the serialized BIR:
    hoist excess on_wait entries onto same-engine NoOp carriers emitted
    immediately before the instruction."""
    import orjson

    orig = nc.to_json_bytes

    def patched(_self=None):
        d = orjson.loads(orig())
        for fn in d.get("functions", []):
            for bb in fn.get("basicblocks", fn.get("blocks", [])):
                insts = bb.get("instructions")
                if insts is None:
                    continue
                out, nctr = [], 0
                for inst in insts:
                    si = inst.get("sync_info")
                    waits = (si or {}).get("on_wait") or []
                    cap = _MAX_WAITS.get(inst.get("opcode"), _DEF_MAX_WAITS)
                    if len(waits) > cap:
                        keep = waits[:cap]
                        extra = waits[cap:]
                        for w in extra:
                            nctr += 1
                            out.append({
                                "debug": inst.get("debug", 0),
                                "engine": inst["engine"],
                                "ins": [], "outs": [],
                                "name": f"{inst['name']}_w{nctr}",
                                "opcode": "NoOp",
                                "sync_info": {"on_wait": [w],
                                              "on_update": []},
                            })
                        si["on_wait"] = keep
                    out.append(inst)
                bb["instructions"] = out
        return orjson.dumps(d)

    nc.to_json_bytes = patched
    return nc


def build_nc(reps=1):
    nc = bass.Bass()
    xT = nc.declare_dram_parameter("xT", [E, S], F16, isOutput=False)
    wqT = nc.declare_dram_parameter("wqT", [E, HG * D], F16, isOutput=False)
    wkT = nc.declare_dram_parameter("wkT", [E, HG * D], F16, isOutput=False)
    wvT = nc.declare_dram_parameter("wvT", [E, HG * D], F16, isOutput=False)
    woT = nc.declare_dram_parameter("woT", [HG * D, E], F16, isOutput=False)
    cs = nc.declare_dram_parameter("cs", [128, S], F16, isOutput=False)
    snS = nc.declare_dram_parameter("snS", [128, S], F16, isOutput=False)
    mask01 = nc.declare_dram_parameter("mask01", [128, 128], F16,
                                       isOutput=False)
    vones = nc.declare_dram_parameter("vones", [128, HG], F16, isOutput=False)
    ones64 = nc.declare_dram_parameter("ones64", [1, D], F16, isOutput=False)
    outT = nc.declare_dram_parameter("outT", [E, S], F16, isOutput=True)

    JORDER = [1, 2, 3, 0]   # process the small j block last -> short tail
    SWAP = [1, 0, 3, 2, 5, 4, 7, 6, 9, 8, 11, 10, 13, 12, 15, 14,
            17, 16, 19, 18, 21, 20, 23, 22, 25, 24, 27, 26, 29, 28, 31, 30]

    with TileContext(nc) as tc, nc.allow_low_precision("fp16 kernel by design"):
      for _rep in range(reps):
        with (
            tc.tile_pool(name="const", bufs=1) as cpool,
            tc.tile_pool(name="qk", bufs=1) as qkpool,
            tc.tile_pool(name="v", bufs=1) as vpool,
            tc.tile_pool(name="pt", bufs=2) as ptpool,
            tc.tile_pool(name="on", bufs=2) as onpool,
            tc.tile_pool(name="sums", bufs=2) as spool,
            tc.tile_pool(name="oc", bufs=3) as ocpool,
        ):
            # ---- inputs: xt spread across SP/DVE/ACT queues for fast start,
            #      weights on ACT, tables on DVE ----
            xt = [cpool.tile([128, S], F16, tag=f"xt{e4}", name=f"xt{e4}")
                  for e4 in range(4)]
            xq = {0: nc.sync, 1: nc.gpsimd, 2: nc.scalar, 3: nc.sync}
            for sh in range(2):
                for e4 in range(4):
                    xq[e4].dma_start(
                        xt[e4][:, sh * 1024:(sh + 1) * 1024],
                        xT[e4 * 128:(e4 + 1) * 128, sh * 1024:(sh + 1) * 1024])
                if sh == 0:
                    wv_t = []
                    for e4 in range(4):
                        t = cpool.tile([128, HG * D], F16, tag=f"wv{e4}",
                                       name=f"wv{e4}")
                        nc.scalar.dma_start(t[:, :],
                                            wvT[e4 * 128:(e4 + 1) * 128, :])
                        wv_t.append(t)
            wq_t, wk_t = [], []
            for nm, dram, lst in (("wk", wkT, wk_t), ("wq", wqT, wq_t)):
                for e4 in range(4):
                    t = cpool.tile([128, HG * D], F16, tag=f"{nm}{e4}",
                                   name=f"{nm}{e4}")
                    nc.scalar.dma_start(t[:, :], dram[e4 * 128:(e4 + 1) * 128, :])
                    lst.append(t)
            wo_t = []
            for f2 in range(2):
                t = cpool.tile([128, E], F16, tag=f"wo{f2}", name=f"wo{f2}")
                nc.scalar.dma_start(t[:, :], woT[f2 * 128:(f2 + 1) * 128, :])
                wo_t.append(t)
            cs16 = cpool.tile([128, S], F16, tag="cs16", name="cs16")
            sn16 = cpool.tile([128, S], F16, tag="sn16", name="sn16")
            nc.gpsimd.dma_start(cs16[:, :], cs[:, :])
            nc.gpsimd.dma_start(sn16[:, :], snS[:, :])
            cs_t = cpool.tile([128, S], F32, tag="cs", name="cs_t")
            sn_t = cpool.tile([128, S], F32, tag="sn", name="sn_t")
            mask_t = cpool.tile([128, 128], F16, tag="mask01", name="mask_t")
            nc.gpsimd.dma_start(mask_t[:, :], mask01[:, :])
            ones_t = cpool.tile([1, D], F16, tag="ones64", name="ones_t")
            nc.gpsimd.dma_start(ones_t[:, :], ones64[:, :])
            nln32 = cpool.tile([64, 1], F32, tag="nln32", name="nln32")
            nc.vector.memset(nln32[:, :], float(-np.log(32.0)))

            # ======== projections + attention, one psum pool ========
            # bank map: duo (128,1024)x3 = 6 banks [q/k proj, scores pairs]
            #           b1  (128,512)x2  = 2 banks [V proj, PV accum, outproj]
            scale = 1.0 / np.sqrt(D)
            v_t = []
            qrot, krot = {}, {}
            onorm = {}
            with tc.tile_pool(name="psA", bufs=1, space="PSUM") as psA:
                def v_proj(ilist, drain):
                    for i in ilist:
                        vt = vpool.tile([128, HG * 65], F16, tag=f"v{i}",
                                        name=f"v{i}")
                        v_t.append(vt)
                        pv = psA.tile([128, 512], F32, tag="b1", bufs=2,
                                      name="pv")
                        for e4 in range(4):
                            nc.tensor.matmul(
                                pv[:, 0:HG * D],
                                xt[e4][:, i * 128:(i + 1) * 128],
                                wv_t[e4][:, :], start=(e4 == 0), stop=(e4 == 3))
                        nc.sync.dma_start(
                            vt[:, :].rearrange("p (h x) -> p h x",
                                               h=HG)[:, :, D:D + 1],
                            vones[:, :, None])
                        drain(
                            vt[:, :].rearrange("p (h x) -> p h x",
                                               h=HG)[:, :, 0:D],
                            pv[:, 0:HG * D].rearrange("p (h d) -> p h d",
                                                      h=HG))

                def project_rope(tgt, wlist, dst, st):
                    rt = qkpool.tile([128, S], F16, tag=f"{tgt}rot{st}",
                                     name=f"{tgt}rot{st}")
                    dst[st] = rt
                    fcol = st * 2 * D
                    for half in range(2):
                        s0 = half * 1024
                        pq = psA.tile([128, 1024], F32, tag="pq",
                                      bufs=1, name="pq")
                        for nb in range(2):
                            c0 = s0 + nb * 512
                            for e4 in range(4):
                                nc.tensor.matmul(
                                    pq[:, nb * 512:(nb + 1) * 512],
                                    wlist[e4][:, fcol:fcol + 128],
                                    xt[e4][:, c0:c0 + 512],
                                    start=(e4 == 0), stop=(e4 == 3))
                        t1 = ptpool.tile([128, 1024], F32, tag="ropetmp",
                                         name="ropetmp")
                        sh_t = ptpool.tile([128, 1024], F32, tag="ropesh",
                                           name="ropesh")
                        nc.vector.stream_shuffle(sh_t[:, :], pq[:, :], SWAP)
                        nc.vector.tensor_mul(
                            t1[:, :], pq[:, :], cs_t[:, s0:s0 + 1024])
                        nc.vector.tensor_mul(
                            sh_t[:, :], sh_t[:, :], sn_t[:, s0:s0 + 1024])
                        nc.vector.tensor_add(
                            rt[:, s0:s0 + 1024], sh_t[:, :], t1[:, :])

                def attention_pass(hp, jlist):
                    """Heads 2hp, 2hp+1 (stack hp).  hp=1 also runs outproj."""
                    for j in jlist:
                        po = [psA.tile([128, 512], F32, tag="b1", bufs=2,
                                       name=f"po{hh}") for hh in range(2)]
                        for i in range(4 * j + 4):
                            sc = psA.tile([128, 1024], F32, tag="duo",
                                          bufs=2, name="sc")
                            r = i - 4 * j
                            offs = 128 * r if r >= 0 else 0
                            partial = r >= 0
                            for hh in range(2):
                                nc.tensor.matmul(
                                    sc[:, hh * 512 + offs:hh * 512 + 512],
                                    krot[hp][hh * D:hh * D + D,
                                             i * 128:(i + 1) * 128],
                                    qrot[hp][hh * D:hh * D + D,
                                             j * 512 + offs:(j + 1) * 512],
                                    start=True, stop=True,
                                    tile_position=(hh * D, 0))
                            pt = ptpool.tile([128, 1024], F16, tag="pt",
                                             bufs=3, name="pt")
                            src = sc[:, :].rearrange("p (h x) -> p h x",
                                                     h=2)[:, :, offs:512]
                            dstv = pt[:, :].rearrange("p (h x) -> p h x",
                                                      h=2)[:, :, offs:512]
                            nc.scalar.activation(
                                dstv, src, mybir.ActivationFunctionType.Exp,
                                scale=float(scale))
                            if partial:
                                # zero the future (upper-tri of the diag
                                # 128-block) on the Pool engine, SBUF-only
                                for hh in range(2):
                                    dv = pt[:, hh * 512 + offs:
                                            hh * 512 + offs + 128]
                                    nc.gpsimd.tensor_mul(dv, dv, mask_t[:, :])
                            for hh in range(2):
                                h = 2 * hp + hh
                                nc.tensor.matmul(
                                    po[hh][0:65, offs:512],
                                    v_t[i][:, h * 65:h * 65 + 65],
                                    pt[:, hh * 512 + offs:hh * 512 + 512],
                                    start=(i == 0), stop=(i == 4 * j + 3))
                        # normalize into the persistent onorm chunk.
                        # Drain po -> SBUF first (frees the b1 psum ring
                        # ~2us earlier than waiting out the norm chain),
                        # then 1/denom via exp(-ln(d)): ACT Ln on the denom
                        # row, a -1s matmul broadcasts -ln(d/32) to 64
                        # partitions, ACT Exp(x - ln32) drains it as 1/d.
                        onj = onpool.tile([128, 512], F16, tag=f"on{j}{hp}",
                                          name=f"on{j}{hp}")
                        onorm[(j, hp)] = onj
                        poS = [spool.tile([65, 512], F32, tag=f"poS{hh}",
                                          name=f"poS{hh}") for hh in range(2)]
                        nld = [spool.tile([1, 512], F16, tag=f"srow{hh}",
                                          name=f"nld{hh}") for hh in range(2)]
                        rbs = [spool.tile([64, 512], F32, tag=f"rb{hh}",
                                          name=f"rb{hh}") for hh in range(2)]
                        for hh in range(2):
                            nc.vector.tensor_copy(poS[hh][:, :], po[hh][0:65, :])
                        for hh in range(2):
                            nc.scalar.activation(
                                nld[hh][:, :], poS[hh][64:65, :],
                                mybir.ActivationFunctionType.Ln,
                                scale=float(1.0 / 32.0))
                        bc = psA.tile([128, 1024], F32, tag="pq",
                                      bufs=1, name="bc")
                        for hh in range(2):
                            nc.tensor.matmul(
                                bc[0:D, hh * 512:hh * 512 + 512],
                                ones_t[:, :], nld[hh][:, :],
                                start=True, stop=True)
                        for hh in range(2):
                            nc.scalar.activation(
                                rbs[hh][:, :],
                                bc[0:D, hh * 512:hh * 512 + 512],
                                mybir.ActivationFunctionType.Exp,
                                bias=nln32[:, :])
                        for hh in range(2):
                            nc.vector.tensor_mul(
                                onj[hh * D:hh * D + D, :],
                                poS[hh][0:D, :], rbs[hh][:, :])
                        if hp == 1:
                            for eb in range(4):
                                pp = psA.tile([128, 512], F32, tag="b1",
                                              bufs=2, name="pp")
                                for f2 in range(2):
                                    nc.tensor.matmul(
                                        pp[:, :],
                                        wo_t[f2][:, eb * 128:
                                                 (eb + 1) * 128],
                                        onorm[(j, f2)][:, :],
                                        start=(f2 == 0), stop=(f2 == 1))
                                oc = ocpool.tile([128, 512], F16, tag="oc",
                                                 name="oc")
                                nc.vector.tensor_copy(oc[:, :], pp[:, :])
                                nc.sync.dma_start(
                                    outT[eb * 128:(eb + 1) * 128,
                                         j * 512:(j + 1) * 512],
                                    oc[:, :])

                v_proj(range(8), nc.scalar.copy)
                nc.vector.tensor_copy(cs_t[:, :], cs16[:, :])
                nc.vector.tensor_copy(sn_t[:, :], sn16[:, :])
                project_rope("k", wk_t, krot, 0)
                project_rope("q", wq_t, qrot, 0)
                attention_pass(0, [1])
                v_proj(range(8, NT), nc.vector.tensor_copy)
                project_rope("k", wk_t, krot, 1)
                attention_pass(0, [2])
                project_rope("q", wq_t, qrot, 1)
                attention_pass(0, [3, 0])
                attention_pass(1, JORDER)
    return _split_excess_waits(nc)


_NC_CACHE = {}


def _get_nc(reps=1):
    if reps not in _NC_CACHE:
        _NC_CACHE[reps] = build_nc(reps)
    return _NC_CACHE[reps]


_RUNNER_CACHE = {}


def _get_runner(nc, n_cores):
    """Clone of bass2jax.run_bass_via_pjrt's multi-core path with the
    jitted callable cached so repeat calls skip retracing."""
    key = id(nc)
    if key in _RUNNER_CACHE:
        return _RUNNER_CACHE[key]
    import jax
    from jax.sharding import Mesh, PartitionSpec
    from jax.experimental.shard_map import shard_map
    from concourse import bass2jax as b2j

    b2j.install_neuronx_cc_hook()
    partition_name = (nc.partition_id_tensor.name
                      if nc.partition_id_tensor else None)
    in_names, out_names, out_avals, zero_outs = [], [], [], []
    for alloc in nc.m.functions[0].allocations:
        if not isinstance(alloc, mybir.MemoryLocationSet):
            continue
        name = alloc.memorylocations[0].name
        if alloc.kind == "ExternalInput":
            if name != partition_name:
                in_names.append(name)
        elif alloc.kind == "ExternalOutput":
            shape = tuple(alloc.tensor_shape)
            dtype = mybir.dt.np(alloc.dtype)
            out_names.append(name)
            out_avals.append(jax.core.ShapedArray(shape, dtype))
            zero_outs.append(np.zeros(shape, dtype))
    n_params = len(in_names)
    n_outs = len(out_avals)
    in_names_all = list(in_names) + list(out_names)
    if partition_name is not None:
        in_names_all.append(partition_name)
    donate = tuple(range(n_params, n_params + n_outs))

    def _body(*args):
        operands = list(args)
        if partition_name is not None:
            operands.append(b2j.partition_id_tensor())
        outs = b2j._bass_exec_p.bind(
            *operands,
            out_avals=tuple(out_avals),
            in_names=tuple(in_names_all),
            out_names=tuple(out_names),
            lowering_input_output_aliases=(),
            sim_require_finite=True,
            sim_require_nnan=True,
            nc=nc,
        )
        return tuple(outs)

    devices = jax.devices()[:n_cores]
    mesh = Mesh(np.asarray(devices), ("core",))
    in_specs = (PartitionSpec("core"),) * (n_params + n_outs)
    out_specs = (PartitionSpec("core"),) * len(out_names)
    sharded = jax.jit(
        shard_map(_body, mesh=mesh, in_specs=in_specs, out_specs=out_specs,
                  check_rep=False),
        donate_argnums=donate, keep_unused=True)

    def run(in_maps):
        gins = [np.concatenate([np.asarray(m[name]) for m in in_maps], axis=0)
                for name in in_names]
        gzeros = [np.concatenate([z] * n_cores, axis=0) for z in zero_outs]
        outs = sharded(*gins, *gzeros)
        res = []
        for c in range(n_cores):
            res.append({})
        for i, name in enumerate(out_names):
            arr = np.asarray(outs[i])
            per = arr.shape[0] // n_cores
            for c in range(n_cores):
                res[c][name] = arr[c * per:(c + 1) * per]
        return res

    _RUNNER_CACHE[key] = run
    return run


def _make_in_maps(x, wqkv_w, wqkv_b, wo_w):
    in_maps = []
    wcache = {}
    for c in range(NCORE):
        g, hg = c // 2, c % 2
        if hg not in wcache:
            wcache[hg] = _host_weights(wqkv_w, wqkv_b, wo_w, hg)
        wd = wcache[hg]
        in_maps.append(dict(
            xT=np.ascontiguousarray(x[g].T.astype(np.float16)),
            wqT=wd["wqT"], wkT=wd["wkT"],
            wvT=wd["wvT"], woT=wd["woT"], cs=wd["cs"], snS=wd["snS"],
            mask01=wd["mask01"], vones=wd["vones"],
            ones64=wd["ones64"],
        ))
    return in_maps


def kernel(layer_idx=None, inputs=None, wqkv_w=None, wqkv_b=None,
           wo_w=None, wo_b=None):
    inputs = np.asarray(inputs, dtype=np.float32)
    wqkv_w = np.asarray(wqkv_w, dtype=np.float32)
    wqkv_b = np.asarray(wqkv_b, dtype=np.float32)
    wo_w = np.asarray(wo_w, dtype=np.float32)
    wo_b = np.asarray(wo_b, dtype=np.float32)
    assert not np.any(wqkv_b), "nonzero wqkv_b not supported by this kernel build"

    x = inputs.reshape(B * V, S, E)
    nc = _get_nc()
    in_maps = _make_in_maps(x, wqkv_w, wqkv_b, wo_w)

    run = _get_runner(nc, NCORE)
    outs = run(in_maps)
    y = np.empty((B * V, S, E), dtype=np.float32)
    for g in range(B * V):
        acc = (outs[2 * g]["outT"].astype(np.float32)
               + outs[2 * g + 1]["outT"].astype(np.float32))   # (E, S)
        y[g] = acc.T
    y += wo_b[None, None, :]
    return y.reshape(B, V, S, E)


# revision 33
# speedup vs baseline: 1.2601x; 1.0475x over previous
"""Trainium2 Bass kernel for BaseMultiheadAttention.

dims: B=1, V=4, S=2048, E=512, H=8, D=64 (head_dim), causal, interleaved RoPE.

Sharding (8 cores): core c -> bv index g = c//2, head-group hg = c%2
(4 heads each).  Each core computes its bv-slice's QKV projection restricted
to its 4 heads, RoPE, causal attention, and a partial output projection
(its heads' wO rows).  Host sums the two partials per bv index.

All matmuls in fp16 (the fp32/float32r path compiles to fp32_mode=HIGH,
~4 PE passes per row; fp16 streams 1 row/cycle at 2.4GHz).  PSUM
accumulation stays fp32 throughout; fp16 DRAM I/O halves DMA traffic.

Device-side layout (per core):
  xT      (512,2048) fp16  x[g]^T, e on partitions (4 chunks of 128),
          DMAs spread over SP/ACT/gpsimd queues for a fast start
  q^T/k^T (128,2048)x2 stacks: two heads per 128 partitions, d on partitions
  RoPE:   qrot = qT*cos + shuffle(qT)*sin' on DVE (fp32 from PSUM, fp16 out)
  scores  S^T (s_k on partitions, s_q free), per (j: s_q 512-block,
          i: s_k 128-tile, h: head) -> psum bank
  exp     one ACT instruction per (i,j) across both heads (strided AP),
          fp16 output; the causal diag 128-block is masked AFTER exp by a
          0/1 upper-tri multiply (SBUF-only, off the PE)
  PV      lhsT = V tile (128, 65) fp16 with a ones column -> row 64 of the
          psum output accumulates the softmax denominator for free
  norm    po drained psum->SBUF early (frees the b1 ring); 1/denom without
          reciprocal: ACT Ln(d/32) -> fp16 row, -1s 1-contraction matmul
          broadcasts -ln(d/32) to 64 partitions, ACT Exp(x - ln32) = 1/d
          (Ln+Exp+Copy share one ACT table set); DVE mul -> fp16 onorm
  outproj woT chunks stationary, O^T moving; psum -> SBUF fp16 -> DRAM outT.
"""

import numpy as np

import concourse.bass as bass
import concourse.mybir as mybir
from concourse.tile import TileContext

# ---- problem dims (hardcoded per the task contract) ----
B, V, S, E, H = 1, 4, 2048, 512, 8
D = E // H            # 64
HG = 4                # heads per core
NCORE = 8
NT = S // 128         # 16 s_k tiles
NJ = S // 512         # 4 s_q blocks
F16 = mybir.dt.float16
F32 = mybir.dt.float32



def _host_tables():
    pos = np.arange(S, dtype=np.float64)
    inv_freq = 1.0 / (10000.0 ** (np.arange(0, D, 2, dtype=np.float64) / D))
    freqs = pos[:, None] * inv_freq[None, :]          # (S, D/2)
    freqs = np.repeat(freqs, 2, axis=-1)              # (S, D) interleaved
    cosT = np.cos(freqs).T.astype(np.float32)         # (D, S)
    sinT = np.sin(freqs).T.astype(np.float32)
    cs = np.concatenate([cosT, cosT], axis=0)         # (128, S) two-head stack
    sn = np.concatenate([sinT, sinT], axis=0)
    # causal keep-mask for the 128-wide diag block: keep cols x >= p
    mask01 = np.triu(np.ones((128, 128), dtype=np.float16))
    return cs, sn, mask01


def _host_weights(wqkv_w, wqkv_b, wo_w, hg):
    """Per-head-group weight slices in the kernel's layouts."""
    heads = [hg * HG + h for h in range(HG)]
    cs, sn, mask01 = _host_tables()
    # feature index inside each qkv block: d*H + h  (d fastest-major: index = d*8+h)
    def rows(block, h):
        d = np.arange(D)
        return block * E + d * H + h
    Wq = np.stack([wqkv_w[rows(0, h)] for h in heads])   # (HG, D, E)
    Wk = np.stack([wqkv_w[rows(1, h)] for h in heads])
    Wv = np.stack([wqkv_w[rows(2, h)] for h in heads])
    def to_T(Wh):   # (HG, D, E) -> (E, HG*D) with col = h*64+d
        return np.transpose(Wh, (2, 0, 1)).reshape(E, HG * D).astype(np.float16)
    wqT = to_T(Wq)
    wkT = to_T(Wk)
    wvT = to_T(Wv)
    # wo rows for this head group: out feature = h_global*64 + d
    woT = np.stack([wo_w[:, (hg * HG + h) * D:(hg * HG + h + 1) * D].T
                    for h in range(HG)])                 # (HG, D, E)
    woT = woT.reshape(HG * D, E).astype(np.float16)      # (256, 512)
    sgn = np.tile(np.array([-1.0, 1.0], np.float32), D // 2)[:, None]
    snS = sn * np.concatenate([sgn, sgn], axis=0)
    return dict(wqT=wqT, wkT=wkT, wvT=wvT, woT=woT,
                cs=cs.astype(np.float16), snS=snS.astype(np.float16),
                mask01=mask01,
                vones=np.ones((128, HG), np.float16),
                ones64=np.full((1, D), -1.0, np.float16))


_MAX_WAITS = {"Matmult": 1}          # per-opcode cap; default below
_DEF_MAX_WAITS = 1


def _split_excess_waits(nc):
    """This walrus build encodes at most ~1 sync-wait per instruction
    (the tail Drain with 3 waits and matmuls with 2 fail codegen with
    'Too many sync wait commands').  Post-process the serialized BIR:
    hoist excess on_wait entries onto same-engine NoOp carriers emitted
    immediately before the instruction."""
    import orjson

    orig = nc.to_json_bytes

    def patched(_self=None):
        d = orjson.loads(orig())
        for fn in d.get("functions", []):
            for bb in fn.get("basicblocks", fn.get("blocks", [])):
                insts = bb.get("instructions")
                if insts is None:
                    continue
                out, nctr = [], 0
                for inst in insts:
                    si = inst.get("sync_info")
                    waits = (si or {}).get("on_wait") or []
                    cap = _MAX_WAITS.get(inst.get("opcode"), _DEF_MAX_WAITS)
                    if len(waits) > cap:
                        keep = waits[:cap]
                        extra = waits[cap:]
                        for w in extra:
                            nctr += 1
                            out.append({
                                "debug": inst.get("debug", 0),
                                "engine": inst["engine"],
                                "ins": [], "outs": [],
                                "name": f"{inst['name']}_w{nctr}",
                                "opcode": "NoOp",
                                "sync_info": {"on_wait": [w],
                                              "on_update": []},
                            })
                        si["on_wait"] = keep
                    out.append(inst)
                bb["instructions"] = out
        return orjson.dumps(d)

    nc.to_json_bytes = patched
    return nc


def build_nc(reps=1):
    nc = bass.Bass()
    xT = nc.declare_dram_parameter("xT", [E, S], F16, isOutput=False)
    wqT = nc.declare_dram_parameter("wqT", [E, HG * D], F16, isOutput=False)
    wkT = nc.declare_dram_parameter("wkT", [E, HG * D], F16, isOutput=False)
    wvT = nc.declare_dram_parameter("wvT", [E, HG * D], F16, isOutput=False)
    woT = nc.declare_dram_parameter("woT", [HG * D, E], F16, isOutput=False)
    cs = nc.declare_dram_parameter("cs", [128, S], F16, isOutput=False)
    snS = nc.declare_dram_parameter("snS", [128, S], F16, isOutput=False)
    mask01 = nc.declare_dram_parameter("mask01", [128, 128], F16,
                                       isOutput=False)
    vones = nc.declare_dram_parameter("vones", [128, HG], F16, isOutput=False)
    ones64 = nc.declare_dram_parameter("ones64", [1, D], F16, isOutput=False)
    outT = nc.declare_dram_parameter("outT", [E, S], F16, isOutput=True)

    JORDER = [1, 2, 3, 0]   # process the small j block last -> short tail
    SWAP = [1, 0, 3, 2, 5, 4, 7, 6, 9, 8, 11, 10, 13, 12, 15, 14,
            17, 16, 19, 18, 21, 20, 23, 22, 25, 24, 27, 26, 29, 28, 31, 30]

    with TileContext(nc) as tc, nc.allow_low_precision("fp16 kernel by design"):
      for _rep in range(reps):
        with (
            tc.tile_pool(name="const", bufs=1) as cpool,
            tc.tile_pool(name="qk", bufs=1) as qkpool,
            tc.tile_pool(name="v", bufs=1) as vpool,
            tc.tile_pool(name="pt", bufs=2) as ptpool,
            tc.tile_pool(name="on", bufs=2) as onpool,
            tc.tile_pool(name="sums", bufs=2) as spool,
            tc.tile_pool(name="oc", bufs=3) as ocpool,
        ):
            # ---- inputs: xt spread across SP/DVE/ACT queues for fast start,
            #      weights on ACT, tables on DVE ----
            xt = [cpool.tile([128, S], F16, tag=f"xt{e4}", name=f"xt{e4}")
                  for e4 in range(4)]
            xq = {0: nc.sync, 1: nc.gpsimd, 2: nc.scalar, 3: nc.sync}
            for sh in range(2):
                for e4 in range(4):
                    xq[e4].dma_start(
                        xt[e4][:, sh * 1024:(sh + 1) * 1024],
                        xT[e4 * 128:(e4 + 1) * 128, sh * 1024:(sh + 1) * 1024])
                if sh == 0:
                    wv_t = []
                    for e4 in range(4):
                        t = cpool.tile([128, HG * D], F16, tag=f"wv{e4}",
                                       name=f"wv{e4}")
                        nc.scalar.dma_start(t[:, :],
                                            wvT[e4 * 128:(e4 + 1) * 128, :])
                        wv_t.append(t)
            wq_t, wk_t = [], []
            for nm, dram, lst in (("wk", wkT, wk_t), ("wq", wqT, wq_t)):
                for e4 in range(4):
                    t = cpool.tile([128, HG * D], F16, tag=f"{nm}{e4}",
                                   name=f"{nm}{e4}")
                    nc.scalar.dma_start(t[:, :], dram[e4 * 128:(e4 + 1) * 128, :])
                    lst.append(t)
            wo_t = []
            for f2 in range(2):
                t = cpool.tile([128, E], F16, tag=f"wo{f2}", name=f"wo{f2}")
                nc.scalar.dma_start(t[:, :], woT[f2 * 128:(f2 + 1) * 128, :])
                wo_t.append(t)
            cs16 = cpool.tile([128, S], F16, tag="cs16", name="cs16")
            sn16 = cpool.tile([128, S], F16, tag="sn16", name="sn16")
            nc.gpsimd.dma_start(cs16[:, :], cs[:, :])
            nc.gpsimd.dma_start(sn16[:, :], snS[:, :])
            cs_t = cpool.tile([128, S], F32, tag="cs", name="cs_t")
            sn_t = cpool.tile([128, S], F32, tag="sn", name="sn_t")
            mask_t = cpool.tile([128, 128], F16, tag="mask01", name="mask_t")
            nc.gpsimd.dma_start(mask_t[:, :], mask01[:, :])
            ones_t = cpool.tile([1, D], F16, tag="ones64", name="ones_t")
            nc.gpsimd.dma_start(ones_t[:, :], ones64[:, :])
            nln32 = cpool.tile([64, 1], F32, tag="nln32", name="nln32")
            nc.vector.memset(nln32[:, :], float(-np.log(32.0)))

            # ======== projections + attention, one psum pool ========
            # bank map: duo (128,1024)x3 = 6 banks [q/k proj, scores pairs]
            #           b1  (128,512)x2  = 2 banks [V proj, PV accum, outproj]
            scale = 1.0 / np.sqrt(D)
            v_t = []
            qrot, krot = {}, {}
            onorm = {}
            with tc.tile_pool(name="psA", bufs=1, space="PSUM") as psA:
                def v_proj(ilist, drain):
                    for i in ilist:
                        vt = vpool.tile([128, HG * 65], F16, tag=f"v{i}",
                                        name=f"v{i}")
                        v_t.append(vt)
                        pv = psA.tile([128, 512], F32, tag="b1", bufs=2,
                                      name="pv")
                        for e4 in range(4):
                            nc.tensor.matmul(
                                pv[:, 0:HG * D],
                                xt[e4][:, i * 128:(i + 1) * 128],
                                wv_t[e4][:, :], start=(e4 == 0), stop=(e4 == 3))
                        nc.sync.dma_start(
                            vt[:, :].rearrange("p (h x) -> p h x",
                                               h=HG)[:, :, D:D + 1],
                            vones[:, :, None])
                        drain(
                            vt[:, :].rearrange("p (h x) -> p h x",
                                               h=HG)[:, :, 0:D],
                            pv[:, 0:HG * D].rearrange("p (h d) -> p h d",
                                                      h=HG))

                def project_rope(tgt, wlist, dst, st):
                    rt = qkpool.tile([128, S], F16, tag=f"{tgt}rot{st}",
                                     name=f"{tgt}rot{st}")
                    dst[st] = rt
                    fcol = st * 2 * D
                    for half in range(2):
                        s0 = half * 1024
                        pq = psA.tile([128, 1024], F32, tag="pq",
                                      bufs=1, name="pq")
                        for nb in range(2):
                            c0 = s0 + nb * 512
                            for e4 in range(4):
                                nc.tensor.matmul(
                                    pq[:, nb * 512:(nb + 1) * 512],
                                    wlist[e4][:, fcol:fcol + 128],
                                    xt[e4][:, c0:c0 + 512],
                                    start=(e4 == 0), stop=(e4 == 3))
                        t1 = ptpool.tile([128, 1024], F32, tag="ropetmp",
                                         name="ropetmp")
                        sh_t = ptpool.tile([128, 1024], F32, tag="ropesh",
                                           name="ropesh")
                        nc.vector.stream_shuffle(sh_t[:, :], pq[:, :], SWAP)
                        nc.vector.tensor_mul(
                            t1[:, :], pq[:, :], cs_t[:, s0:s0 + 1024])
                        nc.vector.tensor_mul(
                            sh_t[:, :], sh_t[:, :], sn_t[:, s0:s0 + 1024])
                        nc.vector.tensor_add(
                            rt[:, s0:s0 + 1024], sh_t[:, :], t1[:, :])

                def attention_pass(hp, jlist):
                    """Heads 2hp, 2hp+1 (stack hp).  hp=1 also runs outproj."""
                    for j in jlist:
                        po = [psA.tile([128, 512], F32, tag="b1", bufs=2,
                                       name=f"po{hh}") for hh in range(2)]
                        for i in range(4 * j + 4):
                            sc = psA.tile([128, 1024], F32, tag="duo",
                                          bufs=2, name="sc")
                            r = i - 4 * j
                            offs = 128 * r if r >= 0 else 0
                            partial = r >= 0
                            for hh in range(2):
                                nc.tensor.matmul(
                                    sc[:, hh * 512 + offs:hh * 512 + 512],
                                    krot[hp][hh * D:hh * D + D,
                                             i * 128:(i + 1) * 128],
                                    qrot[hp][hh * D:hh * D + D,
                                             j * 512 + offs:(j + 1) * 512],
                                    start=True, stop=True,
                                    tile_position=(hh * D, 0))
                            pt = ptpool.tile([128, 1024], F16, tag="pt",
                                             bufs=3, name="pt")
                            src = sc[:, :].rearrange("p (h x) -> p h x",
                                                     h=2)[:, :, offs:512]
                            dstv = pt[:, :].rearrange("p (h x) -> p h x",
                                                      h=2)[:, :, offs:512]
                            nc.scalar.activation(
                                dstv, src, mybir.ActivationFunctionType.Exp,
                                scale=float(scale))
                            if partial:
                                # zero the future (upper-tri of the diag
                                # 128-block); fp16 SBUF-only DVE op runs in
                                # the fast 2x mode
                                for hh in range(2):
                                    dv = pt[:, hh * 512 + offs:
                                            hh * 512 + offs + 128]
                                    nc.vector.tensor_mul(dv, dv, mask_t[:, :])
                            for hh in range(2):
                                h = 2 * hp + hh
                                nc.tensor.matmul(
                                    po[hh][0:65, offs:512],
                                    v_t[i][:, h * 65:h * 65 + 65],
                                    pt[:, hh * 512 + offs:hh * 512 + 512],
                                    start=(i == 0), stop=(i == 4 * j + 3))
                        # normalize into the persistent onorm chunk.
                        # Drain po -> SBUF first (frees the b1 psum ring
                        # ~2us earlier than waiting out the norm chain),
                        # then 1/denom via exp(-ln(d)): ACT Ln on the denom
                        # row, a -1s matmul broadcasts -ln(d/32) to 64
                        # partitions, ACT Exp(x - ln32) drains it as 1/d.
                        onj = onpool.tile([128, 512], F16, tag=f"on{j}{hp}",
                                          name=f"on{j}{hp}")
                        onorm[(j, hp)] = onj
                        poS = [spool.tile([65, 512], F32, tag=f"poS{hh}",
                                          name=f"poS{hh}") for hh in range(2)]
                        nld = [spool.tile([1, 512], F16, tag=f"srow{hh}",
                                          name=f"nld{hh}") for hh in range(2)]
                        rbs = [spool.tile([64, 512], F32, tag=f"rb{hh}",
                                          name=f"rb{hh}") for hh in range(2)]
                        for hh in range(2):
                            nc.vector.tensor_copy(poS[hh][:, :], po[hh][0:65, :])
                        for hh in range(2):
                            nc.scalar.activation(
                                nld[hh][:, :], poS[hh][64:65, :],
                                mybir.ActivationFunctionType.Ln,
                                scale=float(1.0 / 32.0))
                        bc = psA.tile([128, 1024], F32, tag="pq",
                                      bufs=1, name="bc")
                        for hh in range(2):
                            nc.tensor.matmul(
                                bc[0:D, hh * 512:hh * 512 + 512],
                                ones_t[:, :], nld[hh][:, :],
                                start=True, stop=True)
                        for hh in range(2):
                            nc.scalar.activation(
                                rbs[hh][:, :],
                                bc[0:D, hh * 512:hh * 512 + 512],
                                mybir.ActivationFunctionType.Exp,
                                bias=nln32[:, :])
                        for hh in range(2):
                            nc.vector.tensor_mul(
                                onj[hh * D:hh * D + D, :],
                                poS[hh][0:D, :], rbs[hh][:, :])
                        if hp == 1:
                            for eb in range(4):
                                pp = psA.tile([128, 512], F32, tag="b1",
                                              bufs=2, name="pp")
                                for f2 in range(2):
                                    nc.tensor.matmul(
                                        pp[:, :],
                                        wo_t[f2][:, eb * 128:
                                                 (eb + 1) * 128],
                                        onorm[(j, f2)][:, :],
                                        start=(f2 == 0), stop=(f2 == 1))
                                oc = ocpool.tile([128, 512], F16, tag="oc",
                                                 name="oc")
                                nc.vector.tensor_copy(oc[:, :], pp[:, :])
                                nc.sync.dma_start(
                                    outT[eb * 128:(eb + 1) * 128,
                                         j * 512:(j + 1) * 512],
                                    oc[:, :])

                v_proj(range(8), nc.scalar.copy)
                nc.vector.tensor_copy(cs_t[:, :], cs16[:, :])
                nc.vector.tensor_copy(sn_t[:, :], sn16[:, :])
                project_rope("k", wk_t, krot, 0)
                project_rope("q", wq_t, qrot, 0)
                attention_pass(0, [1])
                v_proj(range(8, NT), nc.vector.tensor_copy)
                project_rope("k", wk_t, krot, 1)
                attention_pass(0, [2])
                project_rope("q", wq_t, qrot, 1)
                attention_pass(0, [3, 0])
                attention_pass(1, JORDER)
    return _split_excess_waits(nc)


_NC_CACHE = {}


def _get_nc(reps=1):
    if reps not in _NC_CACHE:
        _NC_CACHE[reps] = build_nc(reps)
    return _NC_CACHE[reps]


_RUNNER_CACHE = {}


def _get_runner(nc, n_cores):
    """Clone of bass2jax.run_bass_via_pjrt's multi-core path with the
    jitted callable cached so repeat calls skip retracing."""
    key = id(nc)
    if key in _RUNNER_CACHE:
        return _RUNNER_CACHE[key]
    import jax
    from jax.sharding import Mesh, PartitionSpec
    from jax.experimental.shard_map import shard_map
    from concourse import bass2jax as b2j

    b2j.install_neuronx_cc_hook()
    partition_name = (nc.partition_id_tensor.name
                      if nc.partition_id_tensor else None)
    in_names, out_names, out_avals, zero_outs = [], [], [], []
    for alloc in nc.m.functions[0].allocations:
        if not isinstance(alloc, mybir.MemoryLocationSet):
            continue
        name = alloc.memorylocations[0].name
        if alloc.kind == "ExternalInput":
            if name != partition_name:
                in_names.append(name)
        elif alloc.kind == "ExternalOutput":
            shape = tuple(alloc.tensor_shape)
            dtype = mybir.dt.np(alloc.dtype)
            out_names.append(name)
            out_avals.append(jax.core.ShapedArray(shape, dtype))
            zero_outs.append(np.zeros(shape, dtype))
    n_params = len(in_names)
    n_outs = len(out_avals)
    in_names_all = list(in_names) + list(out_names)
    if partition_name is not None:
        in_names_all.append(partition_name)
    donate = tuple(range(n_params, n_params + n_outs))

    def _body(*args):
        operands = list(args)
        if partition_name is not None:
            operands.append(b2j.partition_id_tensor())
        outs = b2j._bass_exec_p.bind(
            *operands,
            out_avals=tuple(out_avals),
            in_names=tuple(in_names_all),
            out_names=tuple(out_names),
            lowering_input_output_aliases=(),
            sim_require_finite=True,
            sim_require_nnan=True,
            nc=nc,
        )
        return tuple(outs)

    devices = jax.devices()[:n_cores]
    mesh = Mesh(np.asarray(devices), ("core",))
    in_specs = (PartitionSpec("core"),) * (n_params + n_outs)
    out_specs = (PartitionSpec("core"),) * len(out_names)
    sharded = jax.jit(
        shard_map(_body, mesh=mesh, in_specs=in_specs, out_specs=out_specs,
                  check_rep=False),
        donate_argnums=donate, keep_unused=True)

    def run(in_maps):
        gins = [np.concatenate([np.asarray(m[name]) for m in in_maps], axis=0)
                for name in in_names]
        gzeros = [np.concatenate([z] * n_cores, axis=0) for z in zero_outs]
        outs = sharded(*gins, *gzeros)
        res = []
        for c in range(n_cores):
            res.append({})
        for i, name in enumerate(out_names):
            arr = np.asarray(outs[i])
            per = arr.shape[0] // n_cores
            for c in range(n_cores):
                res[c][name] = arr[c * per:(c + 1) * per]
        return res

    _RUNNER_CACHE[key] = run
    return run


def _make_in_maps(x, wqkv_w, wqkv_b, wo_w):
    in_maps = []
    wcache = {}
    for c in range(NCORE):
        g, hg = c // 2, c % 2
        if hg not in wcache:
            wcache[hg] = _host_weights(wqkv_w, wqkv_b, wo_w, hg)
        wd = wcache[hg]
        in_maps.append(dict(
            xT=np.ascontiguousarray(x[g].T.astype(np.float16)),
            wqT=wd["wqT"], wkT=wd["wkT"],
            wvT=wd["wvT"], woT=wd["woT"], cs=wd["cs"], snS=wd["snS"],
            mask01=wd["mask01"], vones=wd["vones"],
            ones64=wd["ones64"],
        ))
    return in_maps


def kernel(layer_idx=None, inputs=None, wqkv_w=None, wqkv_b=None,
           wo_w=None, wo_b=None):
    inputs = np.asarray(inputs, dtype=np.float32)
    wqkv_w = np.asarray(wqkv_w, dtype=np.float32)
    wqkv_b = np.asarray(wqkv_b, dtype=np.float32)
    wo_w = np.asarray(wo_w, dtype=np.float32)
    wo_b = np.asarray(wo_b, dtype=np.float32)
    assert not np.any(wqkv_b), "nonzero wqkv_b not supported by this kernel build"

    x = inputs.reshape(B * V, S, E)
    nc = _get_nc()
    in_maps = _make_in_maps(x, wqkv_w, wqkv_b, wo_w)

    run = _get_runner(nc, NCORE)
    outs = run(in_maps)
    y = np.empty((B * V, S, E), dtype=np.float32)
    for g in range(B * V):
        acc = (outs[2 * g]["outT"].astype(np.float32)
               + outs[2 * g + 1]["outT"].astype(np.float32))   # (E, S)
        y[g] = acc.T
    y += wo_b[None, None, :]
    return y.reshape(B, V, S, E)


# revision 38
# speedup vs baseline: 1.2656x; 1.0043x over previous
"""Trainium2 Bass kernel for BaseMultiheadAttention.

dims: B=1, V=4, S=2048, E=512, H=8, D=64 (head_dim), causal, interleaved RoPE.

Sharding (8 cores): core c -> bv index g = c//2, head-group hg = c%2
(4 heads each).  Each core computes its bv-slice's QKV projection restricted
to its 4 heads, RoPE, causal attention, and a partial output projection
(its heads' wO rows).  Host sums the two partials per bv index.

All matmuls in fp16 (the fp32/float32r path compiles to fp32_mode=HIGH,
~4 PE passes per row; fp16 streams 1 row/cycle at 2.4GHz).  PSUM
accumulation stays fp32 throughout; fp16 DRAM I/O halves DMA traffic.

Device-side layout (per core):
  xT      (512,2048) fp16  x[g]^T, e on partitions (4 chunks of 128),
          DMAs spread over SP/ACT/gpsimd queues for a fast start
  q^T/k^T (128,2048)x2 stacks: two heads per 128 partitions, d on partitions
  RoPE:   qrot = qT*cos + shuffle(qT)*sin' on DVE (fp32 from PSUM, fp16 out)
  scores  S^T (s_k on partitions, s_q free), per (j: s_q 512-block,
          i: s_k 128-tile, h: head) -> psum bank
  exp     one ACT instruction per (i,j) across both heads (strided AP),
          fp16 output; the causal diag 128-block is masked AFTER exp by a
          0/1 upper-tri multiply (SBUF-only, off the PE)
  PV      lhsT = V tile (128, 65) fp16 with a ones column -> row 64 of the
          psum output accumulates the softmax denominator for free
  norm    po drained psum->SBUF early (frees the b1 ring); 1/denom without
          reciprocal: ACT Ln(d/32) -> fp16 row, -1s 1-contraction matmul
          broadcasts -ln(d/32) to 64 partitions, ACT Exp(x - ln32) = 1/d
          (Ln+Exp+Copy share one ACT table set); DVE mul -> fp16 onorm
  outproj woT chunks stationary, O^T moving; psum -> SBUF fp16 -> DRAM outT.
"""

import numpy as np

import concourse.bass as bass
import concourse.mybir as mybir
from concourse.tile import TileContext

# ---- problem dims (hardcoded per the task contract) ----
B, V, S, E, H = 1, 4, 2048, 512, 8
D = E // H            # 64
HG = 4                # heads per core
NCORE = 8
NT = S // 128         # 16 s_k tiles
NJ = S // 512         # 4 s_q blocks
F16 = mybir.dt.float16
F32 = mybir.dt.float32



def _host_tables():
    pos = np.arange(S, dtype=np.float64)
    inv_freq = 1.0 / (10000.0 ** (np.arange(0, D, 2, dtype=np.float64) / D))
    freqs = pos[:, None] * inv_freq[None, :]          # (S, D/2)
    freqs = np.repeat(freqs, 2, axis=-1)              # (S, D) interleaved
    cosT = np.cos(freqs).T.astype(np.float32)         # (D, S)
    sinT = np.sin(freqs).T.astype(np.float32)
    cs = np.concatenate([cosT, cosT], axis=0)         # (128, S) two-head stack
    sn = np.concatenate([sinT, sinT], axis=0)
    # causal keep-mask for the 128-wide diag block: keep cols x >= p
    mask01 = np.triu(np.ones((128, 128), dtype=np.float16))
    return cs, sn, mask01


def _host_weights(wqkv_w, wqkv_b, wo_w, hg):
    """Per-head-group weight slices in the kernel's layouts."""
    heads = [hg * HG + h for h in range(HG)]
    cs, sn, mask01 = _host_tables()
    # feature index inside each qkv block: d*H + h  (d fastest-major: index = d*8+h)
    def rows(block, h):
        d = np.arange(D)
        return block * E + d * H + h
    Wq = np.stack([wqkv_w[rows(0, h)] for h in heads])   # (HG, D, E)
    Wk = np.stack([wqkv_w[rows(1, h)] for h in heads])
    Wv = np.stack([wqkv_w[rows(2, h)] for h in heads])
    def to_T(Wh):   # (HG, D, E) -> (E, HG*D) with col = h*64+d
        return np.transpose(Wh, (2, 0, 1)).reshape(E, HG * D).astype(np.float16)
    wqT = to_T(Wq)
    wkT = to_T(Wk)
    wvT = to_T(Wv)
    # wo rows for this head group: out feature = h_global*64 + d
    woT = np.stack([wo_w[:, (hg * HG + h) * D:(hg * HG + h + 1) * D].T
                    for h in range(HG)])                 # (HG, D, E)
    woT = woT.reshape(HG * D, E).astype(np.float16)      # (256, 512)
    sgn = np.tile(np.array([-1.0, 1.0], np.float32), D // 2)[:, None]
    snS = sn * np.concatenate([sgn, sgn], axis=0)
    return dict(wqT=wqT, wkT=wkT, wvT=wvT, woT=woT,
                cs=cs.astype(np.float16), snS=snS.astype(np.float16),
                mask01=mask01,
                vones=np.ones((128, HG), np.float16),
                ones64=np.full((1, D), -1.0, np.float16))


_MAX_WAITS = {"Matmult": 1}          # per-opcode cap; default below
_DEF_MAX_WAITS = 1


def _split_excess_waits(nc):
    """This walrus build encodes at most ~1 sync-wait per instruction
    (the tail Drain with 3 waits and matmuls with 2 fail codegen with
    'Too many sync wait commands').  Post-process the serialized BIR:
    hoist excess on_wait entries onto same-engine NoOp carriers emitted
    immediately before the instruction."""
    import orjson

    orig = nc.to_json_bytes

    def patched(_self=None):
        d = orjson.loads(orig())
        for fn in d.get("functions", []):
            for bb in fn.get("basicblocks", fn.get("blocks", [])):
                insts = bb.get("instructions")
                if insts is None:
                    continue
                out, nctr = [], 0
                for inst in insts:
                    si = inst.get("sync_info")
                    waits = (si or {}).get("on_wait") or []
                    cap = _MAX_WAITS.get(inst.get("opcode"), _DEF_MAX_WAITS)
                    if len(waits) > cap:
                        keep = waits[:cap]
                        extra = waits[cap:]
                        for w in extra:
                            nctr += 1
                            out.append({
                                "debug": inst.get("debug", 0),
                                "engine": inst["engine"],
                                "ins": [], "outs": [],
                                "name": f"{inst['name']}_w{nctr}",
                                "opcode": "NoOp",
                                "sync_info": {"on_wait": [w],
                                              "on_update": []},
                            })
                        si["on_wait"] = keep
                    out.append(inst)
                bb["instructions"] = out
        return orjson.dumps(d)

    nc.to_json_bytes = patched
    return nc


def build_nc(reps=1):
    nc = bass.Bass()
    xT = nc.declare_dram_parameter("xT", [E, S], F16, isOutput=False)
    wqT = nc.declare_dram_parameter("wqT", [E, HG * D], F16, isOutput=False)
    wkT = nc.declare_dram_parameter("wkT", [E, HG * D], F16, isOutput=False)
    wvT = nc.declare_dram_parameter("wvT", [E, HG * D], F16, isOutput=False)
    woT = nc.declare_dram_parameter("woT", [HG * D, E], F16, isOutput=False)
    cs = nc.declare_dram_parameter("cs", [128, S], F16, isOutput=False)
    snS = nc.declare_dram_parameter("snS", [128, S], F16, isOutput=False)
    mask01 = nc.declare_dram_parameter("mask01", [128, 128], F16,
                                       isOutput=False)
    vones = nc.declare_dram_parameter("vones", [128, HG], F16, isOutput=False)
    ones64 = nc.declare_dram_parameter("ones64", [1, D], F16, isOutput=False)
    outT = nc.declare_dram_parameter("outT", [E, S], F16, isOutput=True)

    JORDER = [1, 2, 3, 0]   # process the small j block last -> short tail
    SWAP = [1, 0, 3, 2, 5, 4, 7, 6, 9, 8, 11, 10, 13, 12, 15, 14,
            17, 16, 19, 18, 21, 20, 23, 22, 25, 24, 27, 26, 29, 28, 31, 30]

    with TileContext(nc) as tc, nc.allow_low_precision("fp16 kernel by design"):
      for _rep in range(reps):
        with (
            tc.tile_pool(name="const", bufs=1) as cpool,
            tc.tile_pool(name="qk", bufs=1) as qkpool,
            tc.tile_pool(name="v", bufs=1) as vpool,
            tc.tile_pool(name="pt", bufs=2) as ptpool,
            tc.tile_pool(name="on", bufs=2) as onpool,
            tc.tile_pool(name="sums", bufs=2) as spool,
            tc.tile_pool(name="oc", bufs=3) as ocpool,
        ):
            # ---- inputs: xt spread across SP/DVE/ACT queues for fast start,
            #      weights on ACT, tables on DVE ----
            xt = [cpool.tile([128, S], F16, tag=f"xt{e4}", name=f"xt{e4}")
                  for e4 in range(4)]
            xq = {0: nc.sync, 1: nc.gpsimd, 2: nc.scalar, 3: nc.sync}
            for sh in range(2):
                for e4 in range(4):
                    xq[e4].dma_start(
                        xt[e4][:, sh * 1024:(sh + 1) * 1024],
                        xT[e4 * 128:(e4 + 1) * 128, sh * 1024:(sh + 1) * 1024])
                if sh == 0:
                    wv_t = []
                    for e4 in range(4):
                        t = cpool.tile([128, HG * D], F16, tag=f"wv{e4}",
                                       name=f"wv{e4}")
                        nc.scalar.dma_start(t[:, :],
                                            wvT[e4 * 128:(e4 + 1) * 128, :])
                        wv_t.append(t)
            wq_t, wk_t = [], []
            for nm, dram, lst in (("wk", wkT, wk_t), ("wq", wqT, wq_t)):
                for e4 in range(4):
                    t = cpool.tile([128, HG * D], F16, tag=f"{nm}{e4}",
                                   name=f"{nm}{e4}")
                    nc.scalar.dma_start(t[:, :], dram[e4 * 128:(e4 + 1) * 128, :])
                    lst.append(t)
            wo_t = []
            for f2 in range(2):
                t = cpool.tile([128, E], F16, tag=f"wo{f2}", name=f"wo{f2}")
                nc.scalar.dma_start(t[:, :], woT[f2 * 128:(f2 + 1) * 128, :])
                wo_t.append(t)
            cs16 = cpool.tile([128, S], F16, tag="cs16", name="cs16")
            sn16 = cpool.tile([128, S], F16, tag="sn16", name="sn16")
            nc.gpsimd.dma_start(cs16[:, :], cs[:, :])
            nc.gpsimd.dma_start(sn16[:, :], snS[:, :])
            cs_t = cpool.tile([128, S], F32, tag="cs", name="cs_t")
            sn_t = cpool.tile([128, S], F32, tag="sn", name="sn_t")
            mask_t = cpool.tile([128, 128], F16, tag="mask01", name="mask_t")
            nc.gpsimd.dma_start(mask_t[:, :], mask01[:, :])
            ones_t = cpool.tile([1, D], F16, tag="ones64", name="ones_t")
            nc.gpsimd.dma_start(ones_t[:, :], ones64[:, :])
            nln32 = cpool.tile([64, 1], F32, tag="nln32", name="nln32")
            nc.vector.memset(nln32[:, :], float(-np.log(32.0)))

            # ======== projections + attention, one psum pool ========
            # bank map: duo (128,1024)x3 = 6 banks [q/k proj, scores pairs]
            #           b1  (128,512)x2  = 2 banks [V proj, PV accum, outproj]
            scale = 1.0 / np.sqrt(D)
            v_t = []
            qrot, krot = {}, {}
            onorm = {}
            with tc.tile_pool(name="psA", bufs=1, space="PSUM") as psA:
                # Deferred-emission queue: PV and norm/outproj blocks are
                # emitted one step late so the in-order PE queue always has
                # independent matmuls (next scores block / projections) to
                # chew on while ACT runs exp/Ln.  Flush BEFORE emitting any
                # PE work that reuses the b1/pq psum rings the deferred
                # closures release (deadlock-safe points).
                deferred = []

                def flush():
                    while deferred:
                        deferred.pop(0)()

                def v_proj(ilist, drain):
                    for i in ilist:
                        flush()
                        vt = vpool.tile([128, HG * 65], F16, tag=f"v{i}",
                                        name=f"v{i}")
                        v_t.append(vt)
                        pv = psA.tile([128, 512], F32, tag="b1", bufs=2,
                                      name="pv")
                        for e4 in range(4):
                            nc.tensor.matmul(
                                pv[:, 0:HG * D],
                                xt[e4][:, i * 128:(i + 1) * 128],
                                wv_t[e4][:, :], start=(e4 == 0), stop=(e4 == 3))
                        nc.sync.dma_start(
                            vt[:, :].rearrange("p (h x) -> p h x",
                                               h=HG)[:, :, D:D + 1],
                            vones[:, :, None])
                        drain(
                            vt[:, :].rearrange("p (h x) -> p h x",
                                               h=HG)[:, :, 0:D],
                            pv[:, 0:HG * D].rearrange("p (h d) -> p h d",
                                                      h=HG))

                def project_rope(tgt, wlist, dst, st):
                    rt = qkpool.tile([128, S], F16, tag=f"{tgt}rot{st}",
                                     name=f"{tgt}rot{st}")
                    dst[st] = rt
                    fcol = st * 2 * D
                    for half in range(2):
                        flush()
                        s0 = half * 1024
                        pq = psA.tile([128, 1024], F32, tag="pq",
                                      bufs=1, name="pq")
                        for nb in range(2):
                            c0 = s0 + nb * 512
                            for e4 in range(4):
                                nc.tensor.matmul(
                                    pq[:, nb * 512:(nb + 1) * 512],
                                    wlist[e4][:, fcol:fcol + 128],
                                    xt[e4][:, c0:c0 + 512],
                                    start=(e4 == 0), stop=(e4 == 3))
                        t1 = ptpool.tile([128, 1024], F32, tag="ropetmp",
                                         name="ropetmp")
                        sh_t = ptpool.tile([128, 1024], F32, tag="ropesh",
                                           name="ropesh")
                        nc.vector.stream_shuffle(sh_t[:, :], pq[:, :], SWAP)
                        nc.vector.tensor_mul(
                            t1[:, :], pq[:, :], cs_t[:, s0:s0 + 1024])
                        nc.vector.tensor_mul(
                            sh_t[:, :], sh_t[:, :], sn_t[:, s0:s0 + 1024])
                        nc.vector.tensor_add(
                            rt[:, s0:s0 + 1024], sh_t[:, :], t1[:, :])

                def attention_pass(hp, jlist):
                    """Heads 2hp, 2hp+1 (stack hp).  hp=1 also runs outproj."""
                    for j in jlist:
                        po = [psA.tile([128, 512], F32, tag="b1", bufs=2,
                                       name=f"po{hh}") for hh in range(2)]
                        for i in range(4 * j + 4):
                            sc = psA.tile([128, 1024], F32, tag="duo",
                                          bufs=2, name="sc")
                            r = i - 4 * j
                            offs = 128 * r if r >= 0 else 0
                            partial = r >= 0
                            for hh in range(2):
                                nc.tensor.matmul(
                                    sc[:, hh * 512 + offs:hh * 512 + 512],
                                    krot[hp][hh * D:hh * D + D,
                                             i * 128:(i + 1) * 128],
                                    qrot[hp][hh * D:hh * D + D,
                                             j * 512 + offs:(j + 1) * 512],
                                    start=True, stop=True,
                                    tile_position=(hh * D, 0))
                            flush()
                            pt = ptpool.tile([128, 1024], F16, tag="pt",
                                             bufs=3, name="pt")
                            src = sc[:, :].rearrange("p (h x) -> p h x",
                                                     h=2)[:, :, offs:512]
                            dstv = pt[:, :].rearrange("p (h x) -> p h x",
                                                      h=2)[:, :, offs:512]
                            nc.scalar.activation(
                                dstv, src, mybir.ActivationFunctionType.Exp,
                                scale=float(scale))
                            if partial:
                                # zero the future (upper-tri of the diag
                                # 128-block); fp16 SBUF-only DVE op runs in
                                # the fast 2x mode
                                for hh in range(2):
                                    dv = pt[:, hh * 512 + offs:
                                            hh * 512 + offs + 128]
                                    nc.vector.tensor_mul(dv, dv, mask_t[:, :])

                            def pv_step(i=i, offs=offs, pt=pt, po=po, j=j):
                                for hh in range(2):
                                    h = 2 * hp + hh
                                    nc.tensor.matmul(
                                        po[hh][0:65, offs:512],
                                        v_t[i][:, h * 65:h * 65 + 65],
                                        pt[:, hh * 512 + offs:
                                           hh * 512 + 512],
                                        start=(i == 0),
                                        stop=(i == 4 * j + 3))
                            deferred.append(pv_step)
                        # normalize into the persistent onorm chunk
                        # (deferred: emitted during the next j's first
                        # iteration so the Ln/broadcast/Exp chain overlaps
                        # PE work).  po -> SBUF drain first frees the b1
                        # psum ring; 1/denom via exp(-ln(d)): ACT Ln on the
                        # denom row, a -1s matmul broadcasts -ln(d/32) to
                        # 64 partitions, ACT Exp(x - ln32) drains it as 1/d.
                        def norm_step(j=j, po=po):
                            onj = onpool.tile([128, 512], F16,
                                              tag=f"on{j}{hp}",
                                              name=f"on{j}{hp}")
                            onorm[(j, hp)] = onj
                            poS = [spool.tile([65, 512], F32, tag=f"poS{hh}",
                                              name=f"poS{hh}")
                                   for hh in range(2)]
                            nld = [spool.tile([1, 512], F16, tag=f"srow{hh}",
                                              name=f"nld{hh}")
                                   for hh in range(2)]
                            rbs = [spool.tile([64, 512], F32, tag=f"rb{hh}",
                                              name=f"rb{hh}")
                                   for hh in range(2)]
                            for hh in range(2):
                                nc.vector.tensor_copy(poS[hh][:, :],
                                                      po[hh][0:65, :])
                            for hh in range(2):
                                nc.scalar.activation(
                                    nld[hh][:, :], poS[hh][64:65, :],
                                    mybir.ActivationFunctionType.Ln,
                                    scale=float(1.0 / 32.0))
                            bc = psA.tile([128, 1024], F32, tag="pq",
                                          bufs=1, name="bc")
                            for hh in range(2):
                                nc.tensor.matmul(
                                    bc[0:D, hh * 512:hh * 512 + 512],
                                    ones_t[:, :], nld[hh][:, :],
                                    start=True, stop=True)
                            for hh in range(2):
                                nc.scalar.activation(
                                    rbs[hh][:, :],
                                    bc[0:D, hh * 512:hh * 512 + 512],
                                    mybir.ActivationFunctionType.Exp,
                                    bias=nln32[:, :])
                            for hh in range(2):
                                nc.vector.tensor_mul(
                                    onj[hh * D:hh * D + D, :],
                                    poS[hh][0:D, :], rbs[hh][:, :])
                            if hp == 1:
                                for eb in range(4):
                                    pp = psA.tile([128, 512], F32, tag="b1",
                                                  bufs=2, name="pp")
                                    for f2 in range(2):
                                        nc.tensor.matmul(
                                            pp[:, :],
                                            wo_t[f2][:, eb * 128:
                                                     (eb + 1) * 128],
                                            onorm[(j, f2)][:, :],
                                            start=(f2 == 0), stop=(f2 == 1))
                                    oc = ocpool.tile([128, 512], F16,
                                                     tag="oc", name="oc")
                                    nc.vector.tensor_copy(oc[:, :], pp[:, :])
                                    nc.sync.dma_start(
                                        outT[eb * 128:(eb + 1) * 128,
                                             j * 512:(j + 1) * 512],
                                        oc[:, :])
                        deferred.append(norm_step)

                v_proj(range(8), nc.scalar.copy)
                nc.vector.tensor_copy(cs_t[:, :], cs16[:, :])
                nc.vector.tensor_copy(sn_t[:, :], sn16[:, :])
                project_rope("k", wk_t, krot, 0)
                project_rope("q", wq_t, qrot, 0)
                attention_pass(0, [1])
                v_proj(range(8, NT), nc.vector.tensor_copy)
                project_rope("k", wk_t, krot, 1)
                attention_pass(0, [2])
                project_rope("q", wq_t, qrot, 1)
                attention_pass(0, [3, 0])
                attention_pass(1, JORDER)
                flush()
    return _split_excess_waits(nc)


_NC_CACHE = {}


def _get_nc(reps=1):
    if reps not in _NC_CACHE:
        _NC_CACHE[reps] = build_nc(reps)
    return _NC_CACHE[reps]


_RUNNER_CACHE = {}


def _get_runner(nc, n_cores):
    """Clone of bass2jax.run_bass_via_pjrt's multi-core path with the
    jitted callable cached so repeat calls skip retracing."""
    key = id(nc)
    if key in _RUNNER_CACHE:
        return _RUNNER_CACHE[key]
    import jax
    from jax.sharding import Mesh, PartitionSpec
    from jax.experimental.shard_map import shard_map
    from concourse import bass2jax as b2j

    b2j.install_neuronx_cc_hook()
    partition_name = (nc.partition_id_tensor.name
                      if nc.partition_id_tensor else None)
    in_names, out_names, out_avals, zero_outs = [], [], [], []
    for alloc in nc.m.functions[0].allocations:
        if not isinstance(alloc, mybir.MemoryLocationSet):
            continue
        name = alloc.memorylocations[0].name
        if alloc.kind == "ExternalInput":
            if name != partition_name:
                in_names.append(name)
        elif alloc.kind == "ExternalOutput":
            shape = tuple(alloc.tensor_shape)
            dtype = mybir.dt.np(alloc.dtype)
            out_names.append(name)
            out_avals.append(jax.core.ShapedArray(shape, dtype))
            zero_outs.append(np.zeros(shape, dtype))
    n_params = len(in_names)
    n_outs = len(out_avals)
    in_names_all = list(in_names) + list(out_names)
    if partition_name is not None:
        in_names_all.append(partition_name)
    donate = tuple(range(n_params, n_params + n_outs))

    def _body(*args):
        operands = list(args)
        if partition_name is not None:
            operands.append(b2j.partition_id_tensor())
        outs = b2j._bass_exec_p.bind(
            *operands,
            out_avals=tuple(out_avals),
            in_names=tuple(in_names_all),
            out_names=tuple(out_names),
            lowering_input_output_aliases=(),
            sim_require_finite=True,
            sim_require_nnan=True,
            nc=nc,
        )
        return tuple(outs)

    devices = jax.devices()[:n_cores]
    mesh = Mesh(np.asarray(devices), ("core",))
    in_specs = (PartitionSpec("core"),) * (n_params + n_outs)
    out_specs = (PartitionSpec("core"),) * len(out_names)
    sharded = jax.jit(
        shard_map(_body, mesh=mesh, in_specs=in_specs, out_specs=out_specs,
                  check_rep=False),
        donate_argnums=donate, keep_unused=True)

    def run(in_maps):
        gins = [np.concatenate([np.asarray(m[name]) for m in in_maps], axis=0)
                for name in in_names]
        gzeros = [np.concatenate([z] * n_cores, axis=0) for z in zero_outs]
        outs = sharded(*gins, *gzeros)
        res = []
        for c in range(n_cores):
            res.append({})
        for i, name in enumerate(out_names):
            arr = np.asarray(outs[i])
            per = arr.shape[0] // n_cores
            for c in range(n_cores):
                res[c][name] = arr[c * per:(c + 1) * per]
        return res

    _RUNNER_CACHE[key] = run
    return run


def _make_in_maps(x, wqkv_w, wqkv_b, wo_w):
    in_maps = []
    wcache = {}
    for c in range(NCORE):
        g, hg = c // 2, c % 2
        if hg not in wcache:
            wcache[hg] = _host_weights(wqkv_w, wqkv_b, wo_w, hg)
        wd = wcache[hg]
        in_maps.append(dict(
            xT=np.ascontiguousarray(x[g].T.astype(np.float16)),
            wqT=wd["wqT"], wkT=wd["wkT"],
            wvT=wd["wvT"], woT=wd["woT"], cs=wd["cs"], snS=wd["snS"],
            mask01=wd["mask01"], vones=wd["vones"],
            ones64=wd["ones64"],
        ))
    return in_maps


def kernel(layer_idx=None, inputs=None, wqkv_w=None, wqkv_b=None,
           wo_w=None, wo_b=None):
    inputs = np.asarray(inputs, dtype=np.float32)
    wqkv_w = np.asarray(wqkv_w, dtype=np.float32)
    wqkv_b = np.asarray(wqkv_b, dtype=np.float32)
    wo_w = np.asarray(wo_w, dtype=np.float32)
    wo_b = np.asarray(wo_b, dtype=np.float32)
    assert not np.any(wqkv_b), "nonzero wqkv_b not supported by this kernel build"

    x = inputs.reshape(B * V, S, E)
    nc = _get_nc()
    in_maps = _make_in_maps(x, wqkv_w, wqkv_b, wo_w)

    run = _get_runner(nc, NCORE)
    outs = run(in_maps)
    y = np.empty((B * V, S, E), dtype=np.float32)
    for g in range(B * V):
        acc = (outs[2 * g]["outT"].astype(np.float32)
               + outs[2 * g + 1]["outT"].astype(np.float32))   # (E, S)
        y[g] = acc.T
    y += wo_b[None, None, :]
    return y.reshape(B, V, S, E)
